# revision 14
# baseline (speedup 1.0000x reference)
"""Trainium2 Bass kernel for nn_ExactSpectralHead (sparse resonance attention).

Reference (per batch b):  q,k,v = x@W{q,k,v}.T;  s = qk^T/sqrt(C) + bias;
  p = softmax(where(allowed, s, -inf));  out = p@v.

Strategy (8 cores, one batch element per core):
  - "Empty" positions (allowed row AND column are diagonal-only) attend only
    to themselves and nobody attends them => out = v.  Permute so the 1730
    non-empty positions (plus 62 empty pads) occupy slots 0..1791; all 318
    empty positions are served by a dedicated bf16 V projection side-path
    (their output is exactly v, so it needs better precision than fp8).
  - x is shipped fp8-only.  Q/K/V projections all run in fp8 DoubleRow
    (2 output cols/PE-cycle).  V adds a second accumulation pass with the
    fp8 quantization residual of Wv (delta compensation) to kill the
    systematic weight-quantization error.
  - EB = 1+res (exact small ints) shipped as fp16.  p_raw = exp(qk/sqrt(C))
    * EB; normalization deferred to a ones-matmul row-sum after PV.
  - Query blocks of 512 (last 256) x key chunks of 128 so Q/K/V all slice
    one x8 SBUF buffer cleanly.  Host computes per-(chunk,block) active
    column ranges from EB; chunk pairs share a union range.
  - PSUM->SBUF casts run on the Scalar engine (Copy shares the act table
    with Exp -> no table reloads); DVE keeps only the fp16 softmax chain.
  - PE pstate: warm-up matmuls run during the DMA head so the 0.65/1.2/2.4
    GHz ramp completes before the projection stream begins; Q/K psums are
    double-buffered so the PE never waits on a cast.
"""

import sys

sys.path.insert(0, "/opt/trn_rl_repo")

import numpy as np
import ml_dtypes

import concourse.bass as bass
import concourse.tile as tile
import concourse.mybir as mybir

# ----------------------------------------------------------------------------
# Workaround for walrus codegen "Too many sync wait commands" on the
# TileContext tail Drain: split the global-clock sem waits across multiple SP
# NOP instructions instead of attaching them all to the single Drain.
from concourse.vector_clock import ScopedClock, VectorClock


def _split_drain_and_barrier(self, tick_clock, wait_clock):
    import concourse.mybir as _mybir

    nc = self.nc
    gc = tick_clock.global_clock
    n = len(gc)
    for p in range(n):
        t = gc[p]
        if t > 0:
            nop = nc.sync.nop(nofuse=True, hint=f"drain_wait_{p}")
            vc = VectorClock([t if i == p else 0 for i in range(n)])
            wait_clock.add_sem_waits(nop.ins, ScopedClock({None: vc}))

    tail_sem = nc.alloc_semaphore("tile_tail_sem")
    n_signals = 0
    for etype, eng in nc.engines.items():
        if etype == _mybir.EngineType.Pool:
            continue
        eng.drain(fusable=False)
        eng.sem_inc(tail_sem, 1)
        n_signals += 1
    nc.gpsimd.wait_ge(tail_sem, n_signals)
    assert self.sems is not None
    popped = nc._tile_sem_poison_stack.pop()
    assert popped is self._sem_poison
    nc.clear_and_free_semaphores(list(self.sems.allocated().values()))
    nc.gpsimd.sem_clear(range(tail_sem.num, tail_sem.num + 1))


tile.TileContext._drain_and_barrier = _split_drain_and_barrier
# ----------------------------------------------------------------------------


def _split_excess_waits(nc, max_waits=1):
    """Walrus codegen supports only one sem-wait per instruction; hoist excess
    waits onto preceding same-engine NOPs, and replace the slow EventSemaphore
    ops with NoOps carrying the same sync_info."""
    for f in nc.m.functions:
        for bb in f.blocks:
            new = []
            changed = False
            for inst in bb.instructions:
                if isinstance(inst, mybir.InstEventSemaphore):
                    si = inst.sync_info
                    changed = True
                    w = list(si.on_wait) if si else []
                    u = list(si.on_update) if si else []
                    if w:
                        new.append(
                            mybir.InstNoOp(
                                name=f"{inst.name}-wait",
                                engine=inst.engine,
                                bass_nofuse=True,
                                sync_info=mybir.SyncInfo(on_wait=w, on_update=[]),
                            )
                        )
                    new.append(
                        mybir.InstNoOp(
                            name=inst.name,
                            engine=inst.engine,
                            bass_nofuse=True,
                            sync_info=mybir.SyncInfo(on_wait=[], on_update=u),
                        )
                    )
                    continue
                si = inst.sync_info
                waits = list(si.on_wait) if si is not None else []
                if len(waits) > max_waits:
                    changed = True
                    excess, keep = waits[:-max_waits], waits[-max_waits:]
                    for k, w in enumerate(excess):
                        new.append(
                            mybir.InstNoOp(
                                name=f"{inst.name}-w{k}",
                                engine=inst.engine,
                                bass_nofuse=True,
                                sync_info=mybir.SyncInfo(on_wait=[w], on_update=[]),
                            )
                        )
                    inst.sync_info = mybir.SyncInfo(
                        on_wait=keep, on_update=list(si.on_update)
                    )
                new.append(inst)
            if changed:
                bb.instructions = new


B, T, C, H = 8, 2048, 1024, 128
NCORES = 8
SCALE = float(C) ** -0.5
P = 128
NACT = 1792                  # active positions (1730 nonempty + 62 pads)
NKC = NACT // P              # 14 key chunks
QB = (0, 512, 1024, 1536, 1792)   # query block bounds
NJ = 4
NEMPTY = 318                 # empty positions served by bf16 V side-path
NEV = 320                    # side-path projection width (perm cols 1728..2048)
NWARM = 8
BF16 = mybir.dt.bfloat16
FP16 = mybir.dt.float16
FP8 = mybir.dt.float8e4
F32 = mybir.dt.float32
DR = mybir.MatmulPerfMode.DoubleRow
ACT_EXP = mybir.ActivationFunctionType.Exp
ACT_COPY = mybir.ActivationFunctionType.Copy

_nc_cache = None
_sched_cache = None


def _schedule(allowed):
    """Permutation + per-block pair schedule, derived from `allowed`."""
    allowed = np.asarray(allowed, dtype=bool)
    row1 = allowed.sum(1) == 1
    col1 = allowed.sum(0) == 1
    empty = row1 & col1
    nonempty_idx = np.where(~empty)[0]
    empty_idx = np.where(empty)[0]
    assert len(empty_idx) == NEMPTY, len(empty_idx)
    npad = NACT - len(nonempty_idx)
    assert npad >= 0
    full_perm = np.concatenate([nonempty_idx, empty_idx])
    nz = allowed[np.ix_(full_perm[:NACT], full_perm[:NACT])]  # [q, k]

    blocks = []  # per block: list of (i0, i1|None, lo, hi)
    for j in range(NJ):
        q0, q1 = QB[j], QB[j + 1]
        w = q1 - q0
        bq = nz[q0:q1]
        nchunks = q1 // P
        act = []
        for i in range(nchunks):
            colnz = bq[:, i * P:(i + 1) * P].any(axis=1)
            assert colnz.any(), f"inactive chunk {i} in block {j}"
            qlo = int(np.argmax(colnz))
            qhi = w - int(np.argmax(colnz[::-1]))
            act.append((i, qlo, qhi))
        pairs = []
        k = 0
        while k < len(act):
            if k + 1 < len(act):
                lo = min(act[k][1], act[k + 1][1])
                hi = max(act[k][2], act[k + 1][2])
                pairs.append([act[k][0], act[k + 1][0], lo, hi])
                k += 2
            else:
                pairs.append([act[k][0], None, act[k][1], act[k][2]])
                k += 1
        pairs[0][2], pairs[0][3] = 0, w  # first pair covers full width (PSUM init)
        blocks.append([tuple(p) for p in pairs])

    # eb packing offsets (elements per partition), consumption order
    offs = []
    off = 0
    for j in range(NJ):
        boffs = []
        for (i0, i1, lo, hi) in blocks[j]:
            wp = hi - lo
            n = 2 if i1 is not None else 1
            boffs.append(off)
            off += n * wp
        offs.append(boffs)
    return full_perm, blocks, offs, off


def _build_nc(blocks, offs, ebw):
    nc = bass.Bass()
    # host-packed DRAM inputs
    x8t = nc.declare_dram_parameter("x8t", [4, P, 8, 512], FP8, isOutput=False)
    wqk8 = nc.declare_dram_parameter("wqk8", [P, 2, 4, 2, H], FP8, isOutput=False)
    wv8d = nc.declare_dram_parameter("wv8d", [P, 2, 4, 2, H], FP8, isOutput=False)
    # bf16 Wv (tail side-path) + identity packed in one bf16 buffer
    wvi = nc.declare_dram_parameter("wvi", [P, 8 * H], BF16, isOutput=False)
    x16e = nc.declare_dram_parameter("x16e", [P, 8, NEV], BF16, isOutput=False)
    ebt = nc.declare_dram_parameter("ebt", [P, ebw], FP8, isOutput=False)
    outt = nc.declare_dram_parameter("outt", [H, NACT], FP16, isOutput=True)
    outr = nc.declare_dram_parameter("outr", [1, NACT], FP16, isOutput=True)
    outv = nc.declare_dram_parameter("outv", [H, NEMPTY], BF16, isOutput=True)

    with tile.TileContext(nc) as tc:
        with (
            tc.tile_pool(name="const", bufs=1) as const,
            tc.tile_pool(name="qkv_psum", bufs=2, space="PSUM") as qkv_psum,
            tc.tile_pool(name="st_psum", bufs=2, space="PSUM") as st_psum,
            tc.tile_pool(name="ot_psum", bufs=1, space="PSUM") as ot_psum,
            tc.tile_pool(name="rs_psum", bufs=1, space="PSUM") as rs_psum,
            tc.tile_pool(name="ptb", bufs=3) as ptb_pool,
            tc.tile_pool(name="pt", bufs=12) as pt_pool,
            tc.tile_pool(name="ps", bufs=9) as ps_pool,
            tc.tile_pool(name="outs", bufs=2) as out_pool,
        ):
            x8_sb = const.tile([P, 4, 8, 512], FP8, tag="x8", name="x8_sb")
            w8_sb = const.tile([P, 4, 4, 2, H], FP8, tag="w8", name="w8_sb")
            wvi_sb = const.tile([P, 8 * H], BF16, tag="wvi", name="wvi_sb")
            x16e_sb = const.tile([P, 8, NEV], BF16, tag="x16e", name="x16e_sb")
            rsn_sb = const.tile([1, NACT], FP16, tag="rsn", name="rsn_sb")
            eb_sb = const.tile([P, ebw], FP16, tag="eb", name="eb_sb")
            QT_sb = const.tile([P, NACT], BF16, tag="QT", name="QT_sb")
            KT_sb = const.tile([P, NACT], BF16, tag="KT", name="KT_sb")
            VT_sb = const.tile([P, NACT], BF16, tag="VT", name="VT_sb")
            v_sb = const.tile([P, NKC, H], BF16, tag="v", name="v_sb")
            ones_sb = const.tile([P, P], BF16, tag="ones", name="ones_sb")
            warm_sb = const.tile([P, 512], BF16, tag="warm", name="warm_sb")
            nc.vector.memset(ones_sb[:], 1.0)
            nc.vector.memset(warm_sb[:], 0.125)
            wv_sb = wvi_sb[:, 0:8 * H].rearrange("p (c h) -> p c h", c=8)

            # ---- t0 DMA batch.  The gpsimd swdge queue is the fast path
            # (~290 GB/s vs ~110 for hwdge rings): route the critical x8/eb
            # sequence there in consumption order.  eb is stored fp8 in DRAM
            # and cast to fp16 by the swdge during the transfer.  w8qk goes
            # on the (otherwise idle) scalar hwdge ring; the V weights and
            # side-path inputs ride the sync ring.
            with tc.high_priority():
                # scalar hwdge ring is fast but its trigger blocks the scalar
                # engine (~1-2us): only the single most critical piece goes
                # there.  The bulk rides the gpsimd swdge ring in consumption
                # order; the rest on the (slow) sync ring.
                nc.scalar.dma_start(w8_sb[:, 0:2], wqk8[:])
                nc.scalar.dma_start(x8_sb[:, 0, 0:4], x8t[0][:, 0:4])
                nc.sync.dma_start(w8_sb[:, 2:4], wv8d[:])
                nc.sync.dma_start(wvi_sb[:], wvi[:])
                nc.sync.dma_start(x16e_sb[:], x16e[:])
                ebb = [offs[0][0], offs[1][0], offs[2][0], offs[3][0], ebw]
                def ebdma(j, half):
                    m = (ebb[j] + ebb[j + 1]) // 2
                    lo, hi = (ebb[j], m) if half == 0 else (m, ebb[j + 1])
                    nc.gpsimd.dma_start(eb_sb[:, lo:hi], ebt[:, lo:hi])
                nc.gpsimd.dma_start(x8_sb[:, 0, 4:8], x8t[0][:, 4:8])
                ebdma(0, 0)
                ebdma(0, 1)
                nc.gpsimd.dma_start(x8_sb[:, 1], x8t[1])
                ebdma(1, 0)
                ebdma(1, 1)
                nc.gpsimd.dma_start(x8_sb[:, 2], x8t[2])
                ebdma(2, 0)
                nc.gpsimd.dma_start(x8_sb[:, 3, :, 0:256], x8t[3][:, :, 0:256])
                ebdma(2, 1)
                ebdma(3, 0)
                ebdma(3, 1)

            # ---------- emission helpers ----------
            def warm(n=NWARM):
                """PE pstate warm-up matmuls into the (yet unused) rs bank."""
                for _ in range(n):
                    ps = rs_psum.tile([P, 512], F32, tag="rs", name="rs")
                    nc.tensor.matmul(
                        ps[:], lhsT=ones_sb[:], rhs=warm_sb[:],
                        start=True, stop=True, skip_group_check=True,
                    )

            def qkh(j, wsel):
                """Q or K projection for block j (fp8 DoubleRow, 4 c-pairs)."""
                q0, q1 = QB[j], QB[j + 1]
                w = q1 - q0
                dst = QT_sb if wsel == 0 else KT_sb
                ps = qkv_psum.tile([P, 512], F32, tag="qkvps", name="qkvps")
                for pair in range(4):
                    nc.tensor.matmul(
                        ps[:, :w],
                        lhsT=w8_sb[:, wsel, pair],
                        rhs=x8_sb[:, j, 2 * pair:2 * pair + 2, :w],
                        start=(pair == 0),
                        stop=(pair == 3),
                        perf_mode=DR,
                    )
                if j == 0 and wsel == 1:
                    # split K0's cast so score pair 0 (chunks 0,1) starts early
                    nc.scalar.activation(dst[:, 0:256], ps[:, 0:256], ACT_COPY)
                    nc.scalar.activation(dst[:, 256:512], ps[:, 256:512], ACT_COPY)
                else:
                    nc.scalar.activation(dst[:, q0:q1], ps[:, :w], ACT_COPY)

            def vt(vb):
                """V^T projection for 512-col block vb (fp8 DR + dW residual)."""
                w = min(512, NACT - vb * 512)
                ps = qkv_psum.tile([P, 512], F32, tag="qkvps", name="qkvps")
                for i, wsel in enumerate((2, 3)):
                    for pair in range(4):
                        nc.tensor.matmul(
                            ps[:, :w],
                            lhsT=w8_sb[:, wsel, pair],
                            rhs=x8_sb[:, vb, 2 * pair:2 * pair + 2, :w],
                            start=(i == 0 and pair == 0),
                            stop=(i == 1 and pair == 3),
                            perf_mode=DR,
                        )
                nc.vector.tensor_copy(VT_sb[:, vb * 512:vb * 512 + w], ps[:, :w])
                ncha = w // P
                nc.sync.dma_start(
                    v_sb[:, 4 * vb:4 * vb + ncha, :],
                    VT_sb[:, vb * 512:vb * 512 + w],
                    transpose=True,
                )

            def vtail():
                """bf16 V projection of the 318 empty positions (side-path)."""
                ps = qkv_psum.tile([P, 512], F32, tag="qkvps", name="qkvps")
                for c in range(8):
                    nc.tensor.matmul(
                        ps[:, :NEV],
                        lhsT=wv_sb[:, c, :],
                        rhs=x16e_sb[:, c, :],
                        start=(c == 0),
                        stop=(c == 7),
                    )
                otn = out_pool.tile([P, 512], BF16, tag="otn16", name="otn16")
                nc.vector.tensor_copy(otn[:, :NEMPTY], ps[:, 2:2 + NEMPTY])
                nc.gpsimd.dma_start(outv[:], otn[:, :NEMPTY])

            class Blk:
                pass

            def blk_start(j):
                b = Blk()
                b.j = j
                b.q0, b.q1 = QB[j], QB[j + 1]
                b.w = b.q1 - b.q0
                b.pairs = blocks[j]
                b.ot = ot_psum.tile([P, 512], F32, tag="ot", name="ot")
                b.rs = rs_psum.tile([P, 512], F32, tag="rs", name="rs")
                b.pts = []
                b.qsums = []
                b.nt = sum(2 if i1 is not None else 1 for (i0, i1, _, _) in b.pairs)
                b.ti = 0
                return b

            def st(b, p):
                """Score matmuls + exp + EB-mul + pair-sum for pair p."""
                i0, i1, lo, hi = b.pairs[p]
                n = 2 if i1 is not None else 1
                st2 = st_psum.tile([P, 2, 512], F32, tag="st", name="st2")
                for k, i in enumerate((i0, i1)[:n]):
                    nc.tensor.matmul(
                        st2[:, k, lo:hi],
                        lhsT=KT_sb[:, i * P:(i + 1) * P],
                        rhs=QT_sb[:, b.q0 + lo:b.q0 + hi],
                        start=True,
                        stop=True,
                    )
                ptb = ptb_pool.tile([P, 2, 512], FP16, tag="ptb", name="ptb")
                nc.scalar.activation(
                    ptb[:, :n, lo:hi], st2[:, :n, lo:hi], ACT_EXP, scale=SCALE,
                )
                pt = pt_pool.tile([P, 2, 512], FP16, tag="pt", name="pt")
                off = offs[b.j][p]
                w = hi - lo
                ebp = eb_sb[:, off:off + n * w].rearrange("p (n w) -> p n w", n=n)
                nc.vector.tensor_mul(pt[:, :n, lo:hi], ptb[:, :n, lo:hi], ebp)
                b.pts.append(pt)
                if n == 2:
                    qs = ps_pool.tile([P, 512], FP16, tag="psum", name="psum")
                    nc.vector.tensor_add(qs[:, lo:hi], pt[:, 0, lo:hi], pt[:, 1, lo:hi])
                    b.qsums.append((qs[:, lo:hi], lo, hi))
                else:
                    b.qsums.append((pt[:, 0, lo:hi], lo, hi))

            def ot(b, p):
                i0, i1, lo, hi = b.pairs[p]
                n = 2 if i1 is not None else 1
                for k, i in enumerate((i0, i1)[:n]):
                    nc.tensor.matmul(
                        b.ot[:, lo:hi],
                        lhsT=v_sb[:, i, :],
                        rhs=b.pts[p][:, k, lo:hi],
                        start=(b.ti == 0),
                        stop=(b.ti == b.nt - 1),
                        skip_group_check=True,
                    )
                    b.ti += 1

            def rs_all(b):
                nq = len(b.qsums)
                for q, (src, lo, hi) in enumerate(b.qsums):
                    nc.tensor.matmul(
                        b.rs[:, lo:hi],
                        lhsT=ones_sb[:],
                        rhs=src,
                        start=(q == 0),
                        stop=(q == nq - 1),
                        skip_group_check=True,
                    )

            def epi(b):
                otn = out_pool.tile([P, 512], FP16, tag="otn", name="otn")
                if b.j == NJ - 1:
                    # last block: halves split across DVE/scalar, DMAs on the
                    # (by now idle) scalar hwdge ring
                    hw = b.w // 2
                    nc.vector.tensor_copy(otn[:, :hw], b.ot[:, :hw])
                    nc.scalar.activation(rsn_sb[:, b.q0:b.q1], b.rs[0:1, :b.w], ACT_COPY)
                    nc.scalar.dma_start(outt[:, b.q0:b.q0 + hw], otn[:, :hw])
                    nc.scalar.activation(otn[:, hw:b.w], b.ot[:, hw:b.w], ACT_COPY)
                    nc.scalar.dma_start(outr[:], rsn_sb[:])
                    nc.scalar.dma_start(outt[:, b.q0 + hw:b.q1], otn[:, hw:b.w])
                else:
                    nc.vector.tensor_copy(otn[:, :b.w], b.ot[:, :b.w])
                    nc.scalar.activation(rsn_sb[:, b.q0:b.q1], b.rs[0:1, :b.w], ACT_COPY)
                    nc.gpsimd.dma_start(outt[:, b.q0:b.q1], otn[:, :b.w])

            # ---------- global emission ----------
            warm()
            qkh(0, 0)
            qkh(0, 1)

            bs = [blk_start(j) for j in range(NJ)]

            fillers = [
                [lambda: qkh(1, 0), lambda: qkh(1, 1), lambda: vt(0)],
                [lambda: qkh(2, 0), lambda: qkh(2, 1), lambda: vt(1)],
                [lambda: qkh(3, 0), lambda: qkh(3, 1), lambda: vt(2),
                 lambda: vt(3), vtail],
                [],
            ]

            for j in range(NJ):
                b = bs[j]
                prev = bs[j - 1] if j > 0 else None
                fill = fillers[j]
                lag = list(range(len(prev.pairs))) if prev is not None else []
                li = 0
                fi = 0
                own = 0
                for p in range(len(b.pairs)):
                    if j == 0:
                        # block 0: scores first, fillers chase behind
                        st(b, p)
                        continue
                    # pad with one lagged OT pair + one filler per ST pair
                    if li < len(lag):
                        ot(prev, lag[li])
                        li += 1
                    if fi < len(fill):
                        fill[fi]()
                        fi += 1
                    st(b, p)
                    if j == NJ - 1 and p >= 4:
                        # last block: start its own PV early to shrink the tail
                        if li < len(lag):
                            ot(prev, li)
                            li += 1
                        ot(b, own)
                        own += 1
                while li < len(lag):
                    ot(prev, li)
                    li += 1
                if prev is not None:
                    rs_all(prev)
                    epi(prev)
                while fi < len(fill):
                    fill[fi]()
                    fi += 1
                if j == NJ - 1:
                    while own < len(b.pairs):
                        ot(b, own)
                        own += 1
                    rs_all(b)
                    epi(b)

    _split_excess_waits(nc)
    return nc


def kernel(x, Wq, Wk, Wv, resonance_bias, allowed):
    global _nc_cache, _sched_cache
    x = np.asarray(x, dtype=np.float32)
    Wq = np.asarray(Wq, dtype=np.float32)
    Wk = np.asarray(Wk, dtype=np.float32)
    Wv = np.asarray(Wv, dtype=np.float32)
    resonance_bias = np.asarray(resonance_bias, dtype=np.float32)
    allowed = np.asarray(allowed)

    bf16 = ml_dtypes.bfloat16
    fp8 = ml_dtypes.float8_e4m3
    fp16 = np.float16

    if _sched_cache is None:
        _sched_cache = _schedule(allowed)
    full_perm, blocks, offs, ebw = _sched_cache
    if _nc_cache is None:
        _nc_cache = _build_nc(blocks, offs, ebw)
    nc = _nc_cache

    # ---- host packing ----
    EB = np.rint(np.exp(resonance_bias)) * allowed
    EBp = EB[np.ix_(full_perm[:NACT], full_perm[:NACT])]  # [q, k]
    ebT = np.ascontiguousarray(EBp.T)                      # [k, q]
    eb_pack = np.empty((P, ebw), dtype=fp8)
    for j in range(NJ):
        q0 = QB[j]
        for p, (i0, i1, lo, hi) in enumerate(blocks[j]):
            off = offs[j][p]
            w = hi - lo
            qs = slice(q0 + lo, q0 + hi)
            eb_pack[:, off:off + w] = ebT[i0 * P:(i0 + 1) * P, qs].astype(fp8)
            if i1 is not None:
                eb_pack[:, off + w:off + 2 * w] = ebT[i1 * P:(i1 + 1) * P, qs].astype(fp8)

    def packw(w):
        # w: [C, H] f32 (already fp8-representable values)
        return w.reshape(4, 2, P, H).transpose(2, 0, 1, 3)

    WvT = np.ascontiguousarray(Wv.T)
    Wv8 = WvT.astype(fp8).astype(np.float32)
    dWv8 = (WvT - Wv8).astype(fp8).astype(np.float32)
    wqk8 = np.ascontiguousarray(
        np.stack(
            [packw(np.ascontiguousarray(Wq.T)), packw(np.ascontiguousarray(Wk.T))],
            axis=1,
        ).astype(fp8)
    )
    wv8d = np.ascontiguousarray(
        np.stack([packw(Wv8), packw(dWv8)], axis=1).astype(fp8)
    )
    wvi = np.ascontiguousarray(
        WvT.reshape(8, P, H).transpose(1, 0, 2).reshape(P, 8 * H)
    ).astype(bf16)

    in_maps = []
    for b in range(NCORES):
        xT = x[b].T[:, full_perm]                      # [C, T] permuted cols
        xr = xT.reshape(8, P, T)                       # [c, p, t]
        x8t = np.ascontiguousarray(
            xr.reshape(8, P, 4, 512).transpose(2, 1, 0, 3)
        ).astype(fp8)                                  # [4, P, 8, 512]
        x16e = np.ascontiguousarray(
            xr[:, :, T - NEV:T].transpose(1, 0, 2)
        ).astype(bf16)                                 # [P, 8, NEV]
        in_maps.append(
            {
                "x8t": x8t,
                "x16e": x16e,
                "wqk8": wqk8,
                "wv8d": wv8d,
                "wvi": wvi,
                "ebt": eb_pack,
            }
        )

    from concourse import bass2jax

    try:
        results = bass2jax.run_bass_via_pjrt(nc, in_maps, n_cores=NCORES)
    except Exception:
        import time as _time

        _time.sleep(2.0)
        results = bass2jax.run_bass_via_pjrt(nc, in_maps, n_cores=NCORES)

    out = np.empty((B, T, H), dtype=np.float32)
    inv = np.argsort(full_perm)
    nne = T - NEMPTY  # 1730 nonempty
    for b in range(NCORES):
        outt = np.asarray(results[b]["outt"]).astype(np.float32)  # [H, NACT]
        outr_ = np.asarray(results[b]["outr"]).astype(np.float32)  # [1, NACT]
        oattn = (outt / outr_).T                                   # [NACT, H]
        outv = np.asarray(results[b]["outv"]).astype(np.float32)   # [H, NEMPTY]
        full = np.concatenate([oattn[:nne], outv.T], axis=0)  # [T, H] perm order
        out[b] = full[inv]
    return out


# revision 15
# speedup vs baseline: 1.0070x; 1.0070x over previous
"""Trainium2 Bass kernel for nn_ExactSpectralHead (sparse resonance attention).

Reference (per batch b):  q,k,v = x@W{q,k,v}.T;  s = qk^T/sqrt(C) + bias;
  p = softmax(where(allowed, s, -inf));  out = p@v.

Strategy (8 cores, one batch element per core):
  - "Empty" positions (allowed row AND column are diagonal-only) attend only
    to themselves and nobody attends them => out = v.  Permute so the 1730
    non-empty positions (plus 62 empty pads) occupy slots 0..1791; all 318
    empty positions are served by a dedicated bf16 V projection side-path
    (their output is exactly v, so it needs better precision than fp8).
  - x is shipped fp8-only.  Q/K/V projections all run in fp8 DoubleRow
    (2 output cols/PE-cycle).  V adds a second accumulation pass with the
    fp8 quantization residual of Wv (delta compensation) to kill the
    systematic weight-quantization error.
  - EB = 1+res (exact small ints) shipped as fp16.  p_raw = exp(qk/sqrt(C))
    * EB; normalization deferred to a ones-matmul row-sum after PV.
  - Query blocks of 512 (last 256) x key chunks of 128 so Q/K/V all slice
    one x8 SBUF buffer cleanly.  Host computes per-(chunk,block) active
    column ranges from EB; chunk pairs share a union range.
  - PSUM->SBUF casts run on the Scalar engine (Copy shares the act table
    with Exp -> no table reloads); DVE keeps only the fp16 softmax chain.
  - PE pstate: warm-up matmuls run during the DMA head so the 0.65/1.2/2.4
    GHz ramp completes before the projection stream begins; Q/K psums are
    double-buffered so the PE never waits on a cast.
"""

import sys

sys.path.insert(0, "/opt/trn_rl_repo")

import numpy as np
import ml_dtypes

import concourse.bass as bass
import concourse.tile as tile
import concourse.mybir as mybir

# ----------------------------------------------------------------------------
# Workaround for walrus codegen "Too many sync wait commands" on the
# TileContext tail Drain: split the global-clock sem waits across multiple SP
# NOP instructions instead of attaching them all to the single Drain.
from concourse.vector_clock import ScopedClock, VectorClock


def _split_drain_and_barrier(self, tick_clock, wait_clock):
    import concourse.mybir as _mybir

    nc = self.nc
    gc = tick_clock.global_clock
    n = len(gc)
    for p in range(n):
        t = gc[p]
        if t > 0:
            nop = nc.sync.nop(nofuse=True, hint=f"drain_wait_{p}")
            vc = VectorClock([t if i == p else 0 for i in range(n)])
            wait_clock.add_sem_waits(nop.ins, ScopedClock({None: vc}))

    tail_sem = nc.alloc_semaphore("tile_tail_sem")
    n_signals = 0
    for etype, eng in nc.engines.items():
        if etype == _mybir.EngineType.Pool:
            continue
        eng.drain(fusable=False)
        eng.sem_inc(tail_sem, 1)
        n_signals += 1
    nc.gpsimd.wait_ge(tail_sem, n_signals)
    assert self.sems is not None
    popped = nc._tile_sem_poison_stack.pop()
    assert popped is self._sem_poison
    nc.clear_and_free_semaphores(list(self.sems.allocated().values()))
    nc.gpsimd.sem_clear(range(tail_sem.num, tail_sem.num + 1))


tile.TileContext._drain_and_barrier = _split_drain_and_barrier
# ----------------------------------------------------------------------------


def _split_excess_waits(nc, max_waits=1):
    """Walrus codegen supports only one sem-wait per instruction; hoist excess
    waits onto preceding same-engine NOPs, and replace the slow EventSemaphore
    ops with NoOps carrying the same sync_info."""
    for f in nc.m.functions:
        for bb in f.blocks:
            new = []
            changed = False
            for inst in bb.instructions:
                if isinstance(inst, mybir.InstEventSemaphore):
                    si = inst.sync_info
                    changed = True
                    w = list(si.on_wait) if si else []
                    u = list(si.on_update) if si else []
                    if w:
                        new.append(
                            mybir.InstNoOp(
                                name=f"{inst.name}-wait",
                                engine=inst.engine,
                                bass_nofuse=True,
                                sync_info=mybir.SyncInfo(on_wait=w, on_update=[]),
                            )
                        )
                    new.append(
                        mybir.InstNoOp(
                            name=inst.name,
                            engine=inst.engine,
                            bass_nofuse=True,
                            sync_info=mybir.SyncInfo(on_wait=[], on_update=u),
                        )
                    )
                    continue
                si = inst.sync_info
                waits = list(si.on_wait) if si is not None else []
                if len(waits) > max_waits:
                    changed = True
                    excess, keep = waits[:-max_waits], waits[-max_waits:]
                    for k, w in enumerate(excess):
                        new.append(
                            mybir.InstNoOp(
                                name=f"{inst.name}-w{k}",
                                engine=inst.engine,
                                bass_nofuse=True,
                                sync_info=mybir.SyncInfo(on_wait=[w], on_update=[]),
                            )
                        )
                    inst.sync_info = mybir.SyncInfo(
                        on_wait=keep, on_update=list(si.on_update)
                    )
                new.append(inst)
            if changed:
                bb.instructions = new


B, T, C, H = 8, 2048, 1024, 128
NCORES = 8
SCALE = float(C) ** -0.5
P = 128
NACT = 1792                  # active positions (1730 nonempty + 62 pads)
NKC = NACT // P              # 14 key chunks
QB = (0, 512, 1024, 1536, 1792)   # query block bounds
NJ = 4
NEMPTY = 318                 # empty positions served by bf16 V side-path
NEV = 320                    # side-path projection width (perm cols 1728..2048)
NWARM = 12
BF16 = mybir.dt.bfloat16
FP16 = mybir.dt.float16
FP8 = mybir.dt.float8e4
F32 = mybir.dt.float32
DR = mybir.MatmulPerfMode.DoubleRow
ACT_EXP = mybir.ActivationFunctionType.Exp
ACT_COPY = mybir.ActivationFunctionType.Copy

_nc_cache = None
_sched_cache = None


def _schedule(allowed):
    """Permutation + per-block pair schedule, derived from `allowed`."""
    allowed = np.asarray(allowed, dtype=bool)
    row1 = allowed.sum(1) == 1
    col1 = allowed.sum(0) == 1
    empty = row1 & col1
    nonempty_idx = np.where(~empty)[0]
    empty_idx = np.where(empty)[0]
    assert len(empty_idx) == NEMPTY, len(empty_idx)
    npad = NACT - len(nonempty_idx)
    assert npad >= 0
    full_perm = np.concatenate([nonempty_idx, empty_idx])
    nz = allowed[np.ix_(full_perm[:NACT], full_perm[:NACT])]  # [q, k]

    blocks = []  # per block: list of (i0, i1|None, lo, hi)
    for j in range(NJ):
        q0, q1 = QB[j], QB[j + 1]
        w = q1 - q0
        bq = nz[q0:q1]
        nchunks = q1 // P
        act = []
        for i in range(nchunks):
            colnz = bq[:, i * P:(i + 1) * P].any(axis=1)
            assert colnz.any(), f"inactive chunk {i} in block {j}"
            qlo = int(np.argmax(colnz))
            qhi = w - int(np.argmax(colnz[::-1]))
            act.append((i, qlo, qhi))
        pairs = []
        k = 0
        while k < len(act):
            if k + 1 < len(act):
                lo = min(act[k][1], act[k + 1][1])
                hi = max(act[k][2], act[k + 1][2])
                pairs.append([act[k][0], act[k + 1][0], lo, hi])
                k += 2
            else:
                pairs.append([act[k][0], None, act[k][1], act[k][2]])
                k += 1
        pairs[0][2], pairs[0][3] = 0, w  # first pair covers full width (PSUM init)
        blocks.append([tuple(p) for p in pairs])

    # eb packing offsets (elements per partition), consumption order
    offs = []
    off = 0
    for j in range(NJ):
        boffs = []
        for (i0, i1, lo, hi) in blocks[j]:
            wp = hi - lo
            n = 2 if i1 is not None else 1
            boffs.append(off)
            off += n * wp
        offs.append(boffs)
    return full_perm, blocks, offs, off


def _build_nc(blocks, offs, ebw):
    nc = bass.Bass()
    # host-packed DRAM inputs
    x8t = nc.declare_dram_parameter("x8t", [4, P, 8, 512], FP8, isOutput=False)
    wqk8 = nc.declare_dram_parameter("wqk8", [P, 2, 4, 2, H], FP8, isOutput=False)
    wv8d = nc.declare_dram_parameter("wv8d", [P, 2, 4, 2, H], FP8, isOutput=False)
    # bf16 Wv (tail side-path) + identity packed in one bf16 buffer
    wvi = nc.declare_dram_parameter("wvi", [P, 8 * H], BF16, isOutput=False)
    x16e = nc.declare_dram_parameter("x16e", [P, 8, NEV], BF16, isOutput=False)
    ebt = nc.declare_dram_parameter("ebt", [P, ebw], FP8, isOutput=False)
    outt = nc.declare_dram_parameter("outt", [H, NACT], FP16, isOutput=True)
    outr = nc.declare_dram_parameter("outr", [1, NACT], FP16, isOutput=True)
    outv = nc.declare_dram_parameter("outv", [H, NEMPTY], BF16, isOutput=True)

    with tile.TileContext(nc) as tc:
        with (
            tc.tile_pool(name="const", bufs=1) as const,
            tc.tile_pool(name="qkv_psum", bufs=2, space="PSUM") as qkv_psum,
            tc.tile_pool(name="st_psum", bufs=2, space="PSUM") as st_psum,
            tc.tile_pool(name="ot_psum", bufs=1, space="PSUM") as ot_psum,
            tc.tile_pool(name="rs_psum", bufs=1, space="PSUM") as rs_psum,
            tc.tile_pool(name="ptb", bufs=3) as ptb_pool,
            tc.tile_pool(name="pt", bufs=12) as pt_pool,
            tc.tile_pool(name="ps", bufs=9) as ps_pool,
            tc.tile_pool(name="outs", bufs=2) as out_pool,
        ):
            x8_sb = const.tile([P, 4, 8, 512], FP8, tag="x8", name="x8_sb")
            w8_sb = const.tile([P, 4, 4, 2, H], FP8, tag="w8", name="w8_sb")
            wvi_sb = const.tile([P, 8 * H], BF16, tag="wvi", name="wvi_sb")
            x16e_sb = const.tile([P, 8, NEV], BF16, tag="x16e", name="x16e_sb")
            rsn_sb = const.tile([1, NACT], FP16, tag="rsn", name="rsn_sb")
            eb_sb = const.tile([P, ebw], FP16, tag="eb", name="eb_sb")
            QT_sb = const.tile([P, NACT], BF16, tag="QT", name="QT_sb")
            KT_sb = const.tile([P, NACT], BF16, tag="KT", name="KT_sb")
            VT_sb = const.tile([P, NACT], BF16, tag="VT", name="VT_sb")
            v_sb = const.tile([P, NKC, H], BF16, tag="v", name="v_sb")
            ones_sb = const.tile([P, P], BF16, tag="ones", name="ones_sb")
            warm_sb = const.tile([P, 512], BF16, tag="warm", name="warm_sb")
            nc.vector.memset(ones_sb[:], 1.0)
            nc.vector.memset(warm_sb[:], 0.125)
            wv_sb = wvi_sb[:, 0:8 * H].rearrange("p (c h) -> p c h", c=8)

            # ---- t0 DMA batch.  The gpsimd swdge queue is the fast path
            # (~290 GB/s vs ~110 for hwdge rings): route the critical x8/eb
            # sequence there in consumption order.  eb is stored fp8 in DRAM
            # and cast to fp16 by the swdge during the transfer.  w8qk goes
            # on the (otherwise idle) scalar hwdge ring; the V weights and
            # side-path inputs ride the sync ring.
            with tc.high_priority():
                # scalar hwdge ring is fast but its trigger blocks the scalar
                # engine (~1-2us): only the single most critical piece goes
                # there.  The bulk rides the gpsimd swdge ring in consumption
                # order; the rest on the (slow) sync ring.
                nc.scalar.dma_start(w8_sb[:, 0:2], wqk8[:])
                nc.sync.dma_start(w8_sb[:, 2:4], wv8d[:])
                nc.sync.dma_start(wvi_sb[:], wvi[:])
                nc.sync.dma_start(x16e_sb[:], x16e[:])
                ebb = [offs[0][0], offs[1][0], offs[2][0], offs[3][0], ebw]
                def ebdma(j, half):
                    m = (ebb[j] + ebb[j + 1]) // 2
                    lo, hi = (ebb[j], m) if half == 0 else (m, ebb[j + 1])
                    nc.gpsimd.dma_start(eb_sb[:, lo:hi], ebt[:, lo:hi])
                nc.gpsimd.dma_start(x8_sb[:, 0], x8t[0])
                ebdma(0, 0)
                ebdma(0, 1)
                nc.gpsimd.dma_start(x8_sb[:, 1], x8t[1])
                ebdma(1, 0)
                ebdma(1, 1)
                nc.gpsimd.dma_start(x8_sb[:, 2], x8t[2])
                ebdma(2, 0)
                nc.gpsimd.dma_start(x8_sb[:, 3, :, 0:256], x8t[3][:, :, 0:256])
                ebdma(2, 1)
                ebdma(3, 0)
                ebdma(3, 1)

            # ---------- emission helpers ----------
            def warm(n=NWARM):
                """PE pstate warm-up matmuls into the (yet unused) rs bank."""
                for _ in range(n):
                    ps = rs_psum.tile([P, 512], F32, tag="rs", name="rs")
                    nc.tensor.matmul(
                        ps[:], lhsT=ones_sb[:], rhs=warm_sb[:],
                        start=True, stop=True, skip_group_check=True,
                    )

            def qkh(j, wsel):
                """Q or K projection for block j (fp8 DoubleRow, 4 c-pairs)."""
                q0, q1 = QB[j], QB[j + 1]
                w = q1 - q0
                dst = QT_sb if wsel == 0 else KT_sb
                ps = qkv_psum.tile([P, 512], F32, tag="qkvps", name="qkvps")
                for pair in range(4):
                    nc.tensor.matmul(
                        ps[:, :w],
                        lhsT=w8_sb[:, wsel, pair],
                        rhs=x8_sb[:, j, 2 * pair:2 * pair + 2, :w],
                        start=(pair == 0),
                        stop=(pair == 3),
                        perf_mode=DR,
                    )
                if j == 0 and wsel == 1:
                    # split K0's cast so score pair 0 (chunks 0,1) starts early
                    nc.scalar.activation(dst[:, 0:256], ps[:, 0:256], ACT_COPY)
                    nc.scalar.activation(dst[:, 256:512], ps[:, 256:512], ACT_COPY)
                else:
                    nc.scalar.activation(dst[:, q0:q1], ps[:, :w], ACT_COPY)

            def vt(vb):
                """V^T projection for 512-col block vb (fp8 DR + dW residual)."""
                w = min(512, NACT - vb * 512)
                ps = qkv_psum.tile([P, 512], F32, tag="qkvps", name="qkvps")
                for i, wsel in enumerate((2, 3)):
                    for pair in range(4):
                        nc.tensor.matmul(
                            ps[:, :w],
                            lhsT=w8_sb[:, wsel, pair],
                            rhs=x8_sb[:, vb, 2 * pair:2 * pair + 2, :w],
                            start=(i == 0 and pair == 0),
                            stop=(i == 1 and pair == 3),
                            perf_mode=DR,
                        )
                nc.vector.tensor_copy(VT_sb[:, vb * 512:vb * 512 + w], ps[:, :w])
                ncha = w // P
                nc.sync.dma_start(
                    v_sb[:, 4 * vb:4 * vb + ncha, :],
                    VT_sb[:, vb * 512:vb * 512 + w],
                    transpose=True,
                )

            def vtail():
                """bf16 V projection of the 318 empty positions (side-path)."""
                ps = qkv_psum.tile([P, 512], F32, tag="qkvps", name="qkvps")
                for c in range(8):
                    nc.tensor.matmul(
                        ps[:, :NEV],
                        lhsT=wv_sb[:, c, :],
                        rhs=x16e_sb[:, c, :],
                        start=(c == 0),
                        stop=(c == 7),
                    )
                otn = out_pool.tile([P, 512], BF16, tag="otn16", name="otn16")
                nc.vector.tensor_copy(otn[:, :NEMPTY], ps[:, 2:2 + NEMPTY])
                nc.gpsimd.dma_start(outv[:], otn[:, :NEMPTY])

            class Blk:
                pass

            def blk_start(j):
                b = Blk()
                b.j = j
                b.q0, b.q1 = QB[j], QB[j + 1]
                b.w = b.q1 - b.q0
                b.pairs = blocks[j]
                b.ot = ot_psum.tile([P, 512], F32, tag="ot", name="ot")
                b.rs = rs_psum.tile([P, 512], F32, tag="rs", name="rs")
                b.pts = []
                b.qsums = []
                b.nt = sum(2 if i1 is not None else 1 for (i0, i1, _, _) in b.pairs)
                b.ti = 0
                return b

            def st(b, p):
                """Score matmuls + exp + EB-mul + pair-sum for pair p."""
                i0, i1, lo, hi = b.pairs[p]
                n = 2 if i1 is not None else 1
                st2 = st_psum.tile([P, 2, 512], F32, tag="st", name="st2")
                for k, i in enumerate((i0, i1)[:n]):
                    nc.tensor.matmul(
                        st2[:, k, lo:hi],
                        lhsT=KT_sb[:, i * P:(i + 1) * P],
                        rhs=QT_sb[:, b.q0 + lo:b.q0 + hi],
                        start=True,
                        stop=True,
                    )
                ptb = ptb_pool.tile([P, 2, 512], FP16, tag="ptb", name="ptb")
                nc.scalar.activation(
                    ptb[:, :n, lo:hi], st2[:, :n, lo:hi], ACT_EXP, scale=SCALE,
                )
                pt = pt_pool.tile([P, 2, 512], FP16, tag="pt", name="pt")
                off = offs[b.j][p]
                w = hi - lo
                ebp = eb_sb[:, off:off + n * w].rearrange("p (n w) -> p n w", n=n)
                nc.vector.tensor_mul(pt[:, :n, lo:hi], ptb[:, :n, lo:hi], ebp)
                b.pts.append(pt)
                if n == 2:
                    qs = ps_pool.tile([P, 512], FP16, tag="psum", name="psum")
                    nc.vector.tensor_add(qs[:, lo:hi], pt[:, 0, lo:hi], pt[:, 1, lo:hi])
                    b.qsums.append((qs[:, lo:hi], lo, hi))
                else:
                    b.qsums.append((pt[:, 0, lo:hi], lo, hi))

            def ot(b, p):
                i0, i1, lo, hi = b.pairs[p]
                n = 2 if i1 is not None else 1
                for k, i in enumerate((i0, i1)[:n]):
                    nc.tensor.matmul(
                        b.ot[:, lo:hi],
                        lhsT=v_sb[:, i, :],
                        rhs=b.pts[p][:, k, lo:hi],
                        start=(b.ti == 0),
                        stop=(b.ti == b.nt - 1),
                        skip_group_check=True,
                    )
                    b.ti += 1

            def rs_all(b):
                nq = len(b.qsums)
                for q, (src, lo, hi) in enumerate(b.qsums):
                    nc.tensor.matmul(
                        b.rs[:, lo:hi],
                        lhsT=ones_sb[:],
                        rhs=src,
                        start=(q == 0),
                        stop=(q == nq - 1),
                        skip_group_check=True,
                    )

            def epi(b):
                otn = out_pool.tile([P, 512], FP16, tag="otn", name="otn")
                if b.j == NJ - 1:
                    # last block: halves split across DVE/scalar, DMAs on the
                    # (by now idle) scalar hwdge ring
                    hw = b.w // 2
                    nc.vector.tensor_copy(otn[:, :hw], b.ot[:, :hw])
                    nc.scalar.activation(rsn_sb[:, b.q0:b.q1], b.rs[0:1, :b.w], ACT_COPY)
                    nc.scalar.dma_start(outt[:, b.q0:b.q0 + hw], otn[:, :hw])
                    nc.scalar.activation(otn[:, hw:b.w], b.ot[:, hw:b.w], ACT_COPY)
                    nc.scalar.dma_start(outr[:], rsn_sb[:])
                    nc.scalar.dma_start(outt[:, b.q0 + hw:b.q1], otn[:, hw:b.w])
                else:
                    nc.vector.tensor_copy(otn[:, :b.w], b.ot[:, :b.w])
                    nc.scalar.activation(rsn_sb[:, b.q0:b.q1], b.rs[0:1, :b.w], ACT_COPY)
                    nc.gpsimd.dma_start(outt[:, b.q0:b.q1], otn[:, :b.w])

            # ---------- global emission ----------
            warm()
            qkh(0, 0)
            qkh(0, 1)

            bs = [blk_start(j) for j in range(NJ)]

            fillers = [
                [lambda: qkh(1, 0), lambda: qkh(1, 1), lambda: vt(0)],
                [lambda: qkh(2, 0), lambda: qkh(2, 1), lambda: vt(1)],
                [lambda: qkh(3, 0), lambda: qkh(3, 1), lambda: vt(2),
                 lambda: vt(3), vtail],
                [],
            ]

            for j in range(NJ):
                b = bs[j]
                prev = bs[j - 1] if j > 0 else None
                fill = fillers[j]
                lag = list(range(len(prev.pairs))) if prev is not None else []
                li = 0
                fi = 0
                own = 0
                for p in range(len(b.pairs)):
                    if j == 0:
                        # block 0: scores first, fillers chase behind
                        st(b, p)
                        continue
                    # pad with one lagged OT pair + one filler per ST pair
                    if li < len(lag):
                        ot(prev, lag[li])
                        li += 1
                    if fi < len(fill):
                        fill[fi]()
                        fi += 1
                    st(b, p)
                    if j == NJ - 1 and p >= 4:
                        # last block: start its own PV early to shrink the tail
                        if li < len(lag):
                            ot(prev, li)
                            li += 1
                        ot(b, own)
                        own += 1
                while fi < len(fill):
                    fill[fi]()
                    fi += 1
                while li < len(lag):
                    ot(prev, li)
                    li += 1
                if prev is not None:
                    rs_all(prev)
                    epi(prev)
                if j == NJ - 1:
                    while own < len(b.pairs):
                        ot(b, own)
                        own += 1
                    rs_all(b)
                    epi(b)

    _split_excess_waits(nc)
    return nc


def kernel(x, Wq, Wk, Wv, resonance_bias, allowed):
    global _nc_cache, _sched_cache
    x = np.asarray(x, dtype=np.float32)
    Wq = np.asarray(Wq, dtype=np.float32)
    Wk = np.asarray(Wk, dtype=np.float32)
    Wv = np.asarray(Wv, dtype=np.float32)
    resonance_bias = np.asarray(resonance_bias, dtype=np.float32)
    allowed = np.asarray(allowed)

    bf16 = ml_dtypes.bfloat16
    fp8 = ml_dtypes.float8_e4m3
    fp16 = np.float16

    if _sched_cache is None:
        _sched_cache = _schedule(allowed)
    full_perm, blocks, offs, ebw = _sched_cache
    if _nc_cache is None:
        _nc_cache = _build_nc(blocks, offs, ebw)
    nc = _nc_cache

    # ---- host packing ----
    EB = np.rint(np.exp(resonance_bias)) * allowed
    EBp = EB[np.ix_(full_perm[:NACT], full_perm[:NACT])]  # [q, k]
    ebT = np.ascontiguousarray(EBp.T)                      # [k, q]
    eb_pack = np.empty((P, ebw), dtype=fp8)
    for j in range(NJ):
        q0 = QB[j]
        for p, (i0, i1, lo, hi) in enumerate(blocks[j]):
            off = offs[j][p]
            w = hi - lo
            qs = slice(q0 + lo, q0 + hi)
            eb_pack[:, off:off + w] = ebT[i0 * P:(i0 + 1) * P, qs].astype(fp8)
            if i1 is not None:
                eb_pack[:, off + w:off + 2 * w] = ebT[i1 * P:(i1 + 1) * P, qs].astype(fp8)

    def packw(w):
        # w: [C, H] f32 (already fp8-representable values)
        return w.reshape(4, 2, P, H).transpose(2, 0, 1, 3)

    WvT = np.ascontiguousarray(Wv.T)
    Wv8 = WvT.astype(fp8).astype(np.float32)
    dWv8 = (WvT - Wv8).astype(fp8).astype(np.float32)
    wqk8 = np.ascontiguousarray(
        np.stack(
            [packw(np.ascontiguousarray(Wq.T)), packw(np.ascontiguousarray(Wk.T))],
            axis=1,
        ).astype(fp8)
    )
    wv8d = np.ascontiguousarray(
        np.stack([packw(Wv8), packw(dWv8)], axis=1).astype(fp8)
    )
    wvi = np.ascontiguousarray(
        WvT.reshape(8, P, H).transpose(1, 0, 2).reshape(P, 8 * H)
    ).astype(bf16)

    in_maps = []
    for b in range(NCORES):
        xT = x[b].T[:, full_perm]                      # [C, T] permuted cols
        xr = xT.reshape(8, P, T)                       # [c, p, t]
        x8t = np.ascontiguousarray(
            xr.reshape(8, P, 4, 512).transpose(2, 1, 0, 3)
        ).astype(fp8)                                  # [4, P, 8, 512]
        x16e = np.ascontiguousarray(
            xr[:, :, T - NEV:T].transpose(1, 0, 2)
        ).astype(bf16)                                 # [P, 8, NEV]
        in_maps.append(
            {
                "x8t": x8t,
                "x16e": x16e,
                "wqk8": wqk8,
                "wv8d": wv8d,
                "wvi": wvi,
                "ebt": eb_pack,
            }
        )

    from concourse import bass2jax

    try:
        results = bass2jax.run_bass_via_pjrt(nc, in_maps, n_cores=NCORES)
    except Exception:
        import time as _time

        _time.sleep(2.0)
        results = bass2jax.run_bass_via_pjrt(nc, in_maps, n_cores=NCORES)

    out = np.empty((B, T, H), dtype=np.float32)
    inv = np.argsort(full_perm)
    nne = T - NEMPTY  # 1730 nonempty
    for b in range(NCORES):
        outt = np.asarray(results[b]["outt"]).astype(np.float32)  # [H, NACT]
        outr_ = np.asarray(results[b]["outr"]).astype(np.float32)  # [1, NACT]
        oattn = (outt / outr_).T                                   # [NACT, H]
        outv = np.asarray(results[b]["outv"]).astype(np.float32)   # [H, NEMPTY]
        full = np.concatenate([oattn[:nne], outv.T], axis=0)  # [T, H] perm order
        out[b] = full[inv]
    return out
